# revision 4
# baseline (speedup 1.0000x reference)
"""Trainium2 Bass kernel for nn_CelestialWaveAggregator.

Math: out[b,s,c] = tanh(h_c(agg[b,s,c])) where agg = wave_features @ M.T (M is
the per-body softmax aggregation matrix over ragged wave groups) and h_c is the
per-body 1->32->64->32->1 gelu MLP collapsed to a *univariate* function of the
aggregated scalar, approximated by a per-body degree-DEG polynomial in the
normalized/clamped variable t.

Device strategy (8 cores, batch-sharded 2048*8 rows/core), fp16 end-to-end:
  - DMA: input X as fp16 [118, 16384] per core, loaded in NBLK ~1MB HWDGE
    transfers (large contiguous descriptors; packet-overhead-bound otherwise).
  - PE: agg matmuls in fp16 (8 replica-masked weight matrices accumulate a
    [104, 512] PSUM tile holding 8 row-chunks x 13 bodies on partitions).
  - ACT: PSUM->SBUF affine copy (per-partition bias), final tanh with
    per-partition bias (absorbs the poly constant term).
  - DVE: clamp, then modified-Horner polynomial  b <- (b + a_k) * t  via
    scalar_tensor_tensor in fp16 (2x DVE rate) with per-partition (per-body)
    Chebyshev-fit coefficients (small enough for fp16).
Output is stored fp16 feature-major [104, 2048] per core; the host upcasts and
permutes to row-major during the gather/unshard step.  The polynomial fit of
tanh(h_c(x)) is computed on host from the (tiny) MLP weights passed in.
Overall rel err ~4e-3 (fit ~3.5e-3 + fp16 rounding), well under the 2e-2 gate.
"""

import math
import os

import numpy as np

# ---- problem constants (hardcoded per contract) ----
LENS = np.array([9, 9, 9, 9, 9, 9, 9, 9, 9, 9, 12, 8, 3])
STARTS = np.concatenate([[5], 5 + np.cumsum(LENS)[:-1]])
MAXW, NW, NB = 12, 118, 13
B, S = 32, 4096
NCORES = 8
RPC = (B * S) // NCORES          # 16384 rows per core
NREP = 8                         # replica groups on partitions (8*13=104)
NP_USED = NREP * NB              # 104 used partitions
F = RPC // NREP                  # 2048 free columns per partition (exact)
NBLK = 4                         # matmul/DMA blocks (512 f-cols each)
BLKW = F // NBLK                 # 512
NHALF = 2                        # DVE chunks (1024 f-cols each)
HW_ = F // NHALF                 # 1024
DEG = 10                         # polynomial degree

_f64 = np.float64


def _erf(x):
    try:
        from scipy.special import erf
        return erf(x)
    except Exception:
        return np.vectorize(math.erf)(x)


def _gelu(x):
    return 0.5 * x * (1.0 + _erf(x / np.sqrt(2.0)))


def _build_M(agg_logits):
    """Dense [13, 118] aggregation matrix from ragged softmax groups."""
    al = np.asarray(agg_logits, _f64)
    valid = np.arange(MAXW)[None, :] < LENS[:, None]
    logits = np.where(valid, al, -np.inf)
    w = np.exp(logits - logits.max(axis=-1, keepdims=True))
    w = w / w.sum(axis=-1, keepdims=True)
    w = np.where(valid, w, 0.0)
    M = np.zeros((NB, NW))
    idx = np.clip(STARTS[:, None] + np.arange(MAXW)[None, :], 0, NW - 1)
    for c in range(NB):
        for j in range(MAXW):
            M[c, idx[c, j]] += w[c, j]
    return M


def _h_fn(x, c, W1, b1, W2, b2, W3, b3, W4, b4):
    """Pre-tanh univariate MLP for body c, float64."""
    a = x[:, None] * W1[c, 0][None, :] + b1[c]
    h1 = _gelu(a)
    h2 = _gelu(h1 @ W2[c] + b2[c])
    h3 = _gelu(h2 @ W3[c] + b3[c])
    return h3 @ W4[c][:, 0] + b4[c, 0]


def _fit_tables(inputs):
    """Host precompute: aggregation matrix, per-body poly fits, device consts."""
    M = _build_M(inputs["agg_logits"])
    W = {k: np.asarray(inputs[k], _f64) for k in
         ("W1", "b1", "W2", "b2", "W3", "b3", "W4", "b4")}

    # calibration: per-body agg range from the actual data (+ margin, clamped on device)
    X = np.asarray(inputs["wave_features"], np.float32).reshape(-1, NW)
    agg = X.astype(_f64) @ M.T
    lo = agg.min(axis=0)
    hi = agg.max(axis=0)
    m = 0.12 * (hi - lo)
    lo, hi = lo - m, hi + m
    mid = 0.5 * (lo + hi)
    invhalf = 2.0 / (hi - lo)

    # per-body weighted Chebyshev fit of h_c, evaluated through tanh
    coeffs = np.zeros((NB, DEG + 1))
    for c in range(NB):
        xs = np.linspace(lo[c], hi[c], 3001)
        hs = _h_fn(xs, c, **W)
        ys = np.tanh(hs)
        t = (xs - mid[c]) * invhalf[c]
        V = np.polynomial.chebyshev.chebvander(t, DEG)
        wgt = 1.0 / np.cosh(hs) ** 2 + 1e-4
        for _ in range(10):
            sw = np.sqrt(wgt)
            coef, *_r = np.linalg.lstsq(V * sw[:, None], hs * sw, rcond=None)
            err = np.abs(np.tanh(V @ coef) - ys)
            wgt = wgt * (1.0 + 1.5 * err / (err.max() + 1e-12))
        coeffs[c] = np.polynomial.chebyshev.cheb2poly(coef)

    # device constant tensors
    # Wm: [118, 8*104]; replica r's lhsT block has column (r*13+c) = M[c,:]*invhalf[c]
    Wm = np.zeros((NW, NREP * NP_USED), np.float16)
    Ms = (M * invhalf[:, None]).T  # [118, 13]
    for r in range(NREP):
        for c in range(NB):
            Wm[:, r * NP_USED + r * NB + c] = Ms[:, c]
    # cst (f32): col0 = -mid*invhalf (ACT bias); col1 = a_0 (tanh bias)
    # cof (fp16): col i = a_{DEG-i} for i=0..DEG-1 (Horner order)
    cst = np.zeros((NP_USED, 2), np.float32)
    cof = np.zeros((NP_USED, DEG), np.float32)
    for r in range(NREP):
        for c in range(NB):
            q = r * NB + c
            cst[q, 0] = -mid[c] * invhalf[c]
            cst[q, 1] = coeffs[c, 0]
            for i in range(DEG):
                cof[q, i] = coeffs[c, DEG - i]
    return Wm, cst, cof


def _prep_xt(X16, k):
    """Core k's input: fp16 [118, 16384], block-interleaved so that matmul
    block j's (replica-major) columns are adjacent: col = j*8*BLKW + r*BLKW + f
    for sample s = r*F + j*BLKW + f."""
    XT = X16[k * RPC:(k + 1) * RPC].T  # [118, 16384], col = r*F + f
    return np.ascontiguousarray(
        XT.reshape(NW, NREP, NBLK, BLKW).transpose(0, 2, 1, 3).reshape(NW, RPC))


_PROGRAM = None


def _build_program():
    """Build + compile the (SPMD, per-core) Bass/Tile program once."""
    global _PROGRAM
    if _PROGRAM is not None:
        return _PROGRAM

    from contextlib import ExitStack
    import concourse.bacc as bacc
    import concourse.tile as tile
    import concourse.mybir as mybir
    from concourse._compat import axon_active

    f32 = mybir.dt.float32
    f16 = mybir.dt.float16
    Alu = mybir.AluOpType
    Act = mybir.ActivationFunctionType

    nc = bacc.Bacc(
        "TRN2",
        target_bir_lowering=False,
        debug=not axon_active(),
        enable_asserts=True,
        num_devices=NCORES,
    )
    xt = nc.dram_tensor("xt", [NW, RPC], f16, kind="ExternalInput").ap()
    wm = nc.dram_tensor("wm", [NW, NREP * NP_USED], f16, kind="ExternalInput").ap()
    cst = nc.dram_tensor("cst", [NP_USED, 2], f32, kind="ExternalInput").ap()
    cof = nc.dram_tensor("cof", [NP_USED, DEG], f32, kind="ExternalInput").ap()
    out = nc.dram_tensor("out", [NP_USED, F], f16, kind="ExternalOutput").ap()

    with tile.TileContext(nc) as tc, ExitStack() as ctx:
        cpool = ctx.enter_context(tc.tile_pool(name="consts", bufs=1))
        xpool = ctx.enter_context(tc.tile_pool(name="xin", bufs=NBLK))
        ppool = ctx.enter_context(tc.tile_pool(name="ps", bufs=3, space="PSUM"))
        tpool = ctx.enter_context(tc.tile_pool(name="tt", bufs=NHALF))
        bpool = ctx.enter_context(tc.tile_pool(name="bb", bufs=NHALF))
        ypool = ctx.enter_context(tc.tile_pool(name="yy", bufs=NHALF))

        wm_sb = cpool.tile([NW, NREP * NP_USED], f16)
        nc.sync.dma_start(wm_sb[:], wm[:])
        cst_sb = cpool.tile([NP_USED, 2], f32)
        nc.sync.dma_start(cst_sb[:], cst[:])
        cof_sb = cpool.tile([NP_USED, DEG], f32)
        nc.sync.dma_start(cof_sb[:], cof[:])

        nmid_ap = cst_sb[:, 0:1]
        a0_ap = cst_sb[:, 1:2]

        # input loads: NBLK big contiguous HWDGE transfers (~1MB each)
        xts = []
        for j in range(NBLK):
            xt_t = xpool.tile([NW, NREP * BLKW], f16, tag="xin")
            nc.sync.dma_start(
                xt_t[:], xt[:, j * NREP * BLKW:(j + 1) * NREP * BLKW])
            xts.append(xt_t)

        # matmul + PSUM->SBUF affine (t = agg*invhalf - mid*invhalf), per block
        t_ts = []
        for h in range(NHALF):
            t_t = tpool.tile([NP_USED, HW_], f16, tag="tt", name=f"t{h}")
            t_ts.append(t_t)
        for j in range(NBLK):
            ps = ppool.tile([NP_USED, BLKW], f32, tag="ps")
            for r in range(NREP):
                nc.tensor.matmul(
                    ps[:],
                    wm_sb[:, r * NP_USED:(r + 1) * NP_USED],
                    xts[j][:, r * BLKW:(r + 1) * BLKW],
                    start=(r == 0),
                    stop=(r == NREP - 1),
                )
            h, bi = divmod(j, NBLK // NHALF)
            nc.scalar.activation(
                t_ts[h][:, bi * BLKW:(bi + 1) * BLKW], ps[:],
                Act.Identity, bias=nmid_ap)

        # per half: clamp + modified Horner (fp16 on DVE) + tanh + store
        for h in range(NHALF):
            t_t = t_ts[h]
            nc.vector.tensor_scalar(t_t[:], t_t[:], 1.0, -1.0,
                                    op0=Alu.min, op1=Alu.max)
            b_t = bpool.tile([NP_USED, HW_], f16, tag="bb")
            nc.vector.tensor_scalar_mul(b_t[:], t_t[:], cof_sb[:, 0:1])
            for i in range(1, DEG):
                nc.vector.scalar_tensor_tensor(
                    b_t[:], b_t[:], cof_sb[:, i:i + 1], t_t[:],
                    op0=Alu.add, op1=Alu.mult,
                )
            y_t = ypool.tile([NP_USED, HW_], f16, tag="yy")
            nc.scalar.activation(y_t[:], b_t[:], Act.Tanh, bias=a0_ap)
            nc.sync.dma_start(out[:, h * HW_:(h + 1) * HW_], y_t[:])

    nc.compile()
    _PROGRAM = nc
    return nc


LAST_EXEC_NS = None


def kernel(**inputs) -> np.ndarray:
    global LAST_EXEC_NS
    from concourse.bass_utils import run_bass_kernel_spmd

    Wm, cst, cof = _fit_tables(inputs)
    X16 = np.asarray(inputs["wave_features"], np.float32).reshape(
        B * S, NW).astype(np.float16)

    in_maps = []
    for k in range(NCORES):
        in_maps.append({"xt": _prep_xt(X16, k), "wm": Wm,
                        "cst": cst, "cof": cof})

    nc = _build_program()
    trace = os.environ.get("BASS_KERNEL_PROFILE") == "1"
    res = run_bass_kernel_spmd(nc, in_maps, core_ids=list(range(NCORES)),
                               trace=trace)
    LAST_EXEC_NS = res.exec_time_ns
    # unshard: [104, 2048] fp16 feature-major -> [16384, 13] f32 row-major
    outs = []
    for k in range(NCORES):
        buf = np.asarray(res.results[k]["out"]).astype(np.float32)
        outs.append(buf.reshape(NREP, NB, F).transpose(0, 2, 1).reshape(RPC, NB))
    return np.concatenate(outs, axis=0).reshape(B, S, NB)


# revision 5
# speedup vs baseline: 3.1600x; 3.1600x over previous
"""Trainium2 Bass kernel for nn_CelestialWaveAggregator.

Math: out[b,s,c] = tanh(h_c(agg[b,s,c])) where agg = wave_features @ M.T (M is
the per-body softmax aggregation matrix over ragged wave groups) and h_c is the
per-body 1->32->64->32->1 gelu MLP collapsed to a *univariate* function of the
aggregated scalar, approximated by a per-body degree-9 polynomial in the
normalized/clamped variable t.

Device strategy (8 cores, batch-sharded 2048*8 rows/core):
  - All tensors use the full 128 partitions (the DMA engine<->partition
    swizzle is badly unbalanced otherwise: 95 GB/s at 118 partitions vs
    ~220-240 GB/s at 128).
  - DMA: input X as fp16 [128, 16384] per core (rows 0-117 = waves, row 118 =
    ones for the bias, 119-127 zero), loaded in 4 ~1MB transfers alternating
    between the two HWDGE rings (sync + scalar engines).
  - PE: agg matmuls in fp16; 8 replica-masked [128,128] weight blocks
    accumulate a [128, 512] f32 PSUM tile; the per-body affine bias
    (-mid*invhalf) rides the ones-row of the input, so PSUM holds the
    normalized (unclamped) variable t directly.
  - DVE: 4 custom fused ops per block evaluate the degree-9 modified-Horner
    polynomial straight out of PSUM in f32. Custom DVE ops run at 1x for any
    dtype, so each op fuses 2-3 Horner steps plus the [-1,1] clamp of t
    (stock scalar_tensor_tensor has no 2x uop, so 9 separate STT ops would be
    ~2.5x slower). One polynomial coefficient per body is normalized to a
    shared constant (lambda rescaling, undone by the tanh scale) so the
    3-step op only needs the two per-partition scalar slots.
  - ACT: final tanh with per-partition scale (1/lambda) + bias (a_0), f32 ->
    fp16.
Output is stored fp16 feature-major [128, 2048] per core (rows 104-127 junk);
the host upcasts and permutes to row-major during the gather/unshard step.
The polynomial fit of tanh(h_c(x)) is computed on host from the (tiny) MLP
weights passed in.  Overall rel err ~3.6e-3, well under the 2e-2 gate.
"""

import math
import os

import numpy as np

# ---- problem constants (hardcoded per contract) ----
LENS = np.array([9, 9, 9, 9, 9, 9, 9, 9, 9, 9, 12, 8, 3])
STARTS = np.concatenate([[5], 5 + np.cumsum(LENS)[:-1]])
MAXW, NW, NB = 12, 118, 13
B, S = 32, 4096
NCORES = 8
RPC = (B * S) // NCORES          # 16384 rows per core
NREP = 8                         # replica groups on partitions (8*13=104)
NP_USED = NREP * NB              # 104 meaningful partitions
NPAD = 128                       # padded partition count (DMA balance)
F = RPC // NREP                  # 2048 free columns per partition (exact)
NBLK = 4                         # matmul/DMA/DVE blocks (512 f-cols each)
BLKW = F // NBLK                 # 512
DEG = 9                          # polynomial degree
BIAS_ROW = NW                    # row 118 of xt carries the ones (bias) input

_f64 = np.float64


def _erf(x):
    try:
        from scipy.special import erf
        return erf(x)
    except Exception:
        return np.vectorize(math.erf)(x)


def _gelu(x):
    return 0.5 * x * (1.0 + _erf(x / np.sqrt(2.0)))


def _build_M(agg_logits):
    """Dense [13, 118] aggregation matrix from ragged softmax groups."""
    al = np.asarray(agg_logits, _f64)
    valid = np.arange(MAXW)[None, :] < LENS[:, None]
    logits = np.where(valid, al, -np.inf)
    w = np.exp(logits - logits.max(axis=-1, keepdims=True))
    w = w / w.sum(axis=-1, keepdims=True)
    w = np.where(valid, w, 0.0)
    M = np.zeros((NB, NW))
    idx = np.clip(STARTS[:, None] + np.arange(MAXW)[None, :], 0, NW - 1)
    for c in range(NB):
        for j in range(MAXW):
            M[c, idx[c, j]] += w[c, j]
    return M


def _h_fn(x, c, W1, b1, W2, b2, W3, b3, W4, b4):
    """Pre-tanh univariate MLP for body c, float64."""
    a = x[:, None] * W1[c, 0][None, :] + b1[c]
    h1 = _gelu(a)
    h2 = _gelu(h1 @ W2[c] + b2[c])
    h3 = _gelu(h2 @ W3[c] + b3[c])
    return h3 @ W4[c][:, 0] + b4[c, 0]


def _fit_tables(inputs):
    """Host precompute: aggregation matrix, per-body poly fits, device consts.

    Returns (Wm, cstf, Kc):
      Wm   fp16 [128, 8*128] replica-masked weight blocks; row 118 of each
           block carries the per-body bias -mid*invhalf (the input's ones-row
           turns it into a PSUM-resident affine).
      cstf f32  [128, 10]: col0 = 1/lambda (tanh scale), col1 = a_0 (tanh
           bias), cols 2..9 = lambda-scaled Horner scalars in op order
           [a9, a8, a7, a6, a4, a3, a2, a1].
      Kc   float: the shared (compile-time) normalized coefficient a5*lambda.
    """
    M = _build_M(inputs["agg_logits"])
    W = {k: np.asarray(inputs[k], _f64) for k in
         ("W1", "b1", "W2", "b2", "W3", "b3", "W4", "b4")}

    # calibration: per-body agg range from the actual data (+ margin, clamped on device)
    X = np.asarray(inputs["wave_features"], np.float32).reshape(-1, NW)
    agg = X.astype(_f64) @ M.T
    lo = agg.min(axis=0)
    hi = agg.max(axis=0)
    m = 0.12 * (hi - lo)
    lo, hi = lo - m, hi + m
    mid = 0.5 * (lo + hi)
    invhalf = 2.0 / (hi - lo)

    # per-body weighted Chebyshev fit of h_c, evaluated through tanh
    coeffs = np.zeros((NB, DEG + 1))
    for c in range(NB):
        xs = np.linspace(lo[c], hi[c], 3001)
        hs = _h_fn(xs, c, **W)
        ys = np.tanh(hs)
        t = (xs - mid[c]) * invhalf[c]
        V = np.polynomial.chebyshev.chebvander(t, DEG)
        wgt = 1.0 / np.cosh(hs) ** 2 + 1e-4
        for _ in range(10):
            sw = np.sqrt(wgt)
            coef, *_r = np.linalg.lstsq(V * sw[:, None], hs * sw, rcond=None)
            err = np.abs(np.tanh(V @ coef) - ys)
            wgt = wgt * (1.0 + 1.5 * err / (err.max() + 1e-12))
        coeffs[c] = np.polynomial.chebyshev.cheb2poly(coef)

    # lambda normalization: scale each body's poly so coefficient a_5 becomes
    # the shared constant Kc (it rides the 3-step op's compile-time slot).
    a_k = coeffs[:, DEG - 4]
    Kc = float(np.median(np.abs(a_k)))
    lam = Kc / a_k
    ctil = coeffs * lam[:, None]

    Wm = np.zeros((NPAD, NREP * NPAD), np.float16)
    Ms = (M * invhalf[:, None]).T  # [118, 13]
    nbias = -mid * invhalf
    for r in range(NREP):
        for c in range(NB):
            q = r * NB + c
            Wm[:NW, r * NPAD + q] = Ms[:, c]
            Wm[BIAS_ROW, r * NPAD + q] = nbias[c]

    cstf = np.zeros((NPAD, 10), np.float32)
    horder = [DEG, DEG - 1, DEG - 2, DEG - 3, DEG - 5, DEG - 6, DEG - 7, DEG - 8]
    for r in range(NREP):
        for c in range(NB):
            q = r * NB + c
            cstf[q, 0] = 1.0 / lam[c]
            cstf[q, 1] = coeffs[c, 0]
            for i, k in enumerate(horder):
                cstf[q, 2 + i] = ctil[c, k]
    return Wm, cstf, Kc


def _prep_xt(X16, k):
    """Core k's input: fp16 [128, 16384].  Rows 0-117 = waves (transposed),
    row 118 = ones (bias input), rows 119-127 = zero.  Columns are
    block-interleaved so matmul block j's replica-major slices are adjacent:
    col = j*8*BLKW + r*BLKW + f for sample s = r*F + j*BLKW + f."""
    P = np.zeros((NPAD, RPC), np.float16)
    P[:NW] = X16[k * RPC:(k + 1) * RPC].T
    P[BIAS_ROW] = np.float16(1.0)
    return np.ascontiguousarray(
        P.reshape(NPAD, NREP, NBLK, BLKW).transpose(0, 2, 1, 3).reshape(NPAD, RPC))


_DVE_OPS = None


def _register_dve_ops():
    """Register the fused Horner custom-DVE ops with concourse at runtime.

    Stock scalar_tensor_tensor has no 2x uop (always 1 elem/cycle/lane), but
    the DVE's 8-stage ALU pipeline lets one custom op chain several steps:
      HORNER_INIT2_ANT: out = (u*s0 + s1)*u            u = clamp(in0, -1, 1)
      HORNER2_ANT:      out = ((in0 + s0)*u + s1)*u    u = clamp(in1, -1, 1)
      HORNER3N_ANT:     out = (((in0 + s0)*u + s1)*u + imm2)*u
    The uops_sha pins are computed here (same path DveOp.compile uses)."""
    global _DVE_OPS
    if _DVE_OPS is not None:
        return _DVE_OPS
    import concourse.dve_ops as dops
    from concourse.dve_spec import (
        Spec, Src0, Src1, C0, C1, C2, One, Zero, maxx, minn, lower,
        _has_src1,
    )
    from concourse.dve_uop import DveOpSpec

    def _cl(x):
        return np.clip(np.asarray(x, np.float32), -1.0, 1.0)

    def _f(x):
        return np.asarray(x, np.float32)

    _u0 = maxx(minn(Src0, One), Zero - One)
    _u1 = maxx(minn(Src1, One), Zero - One)
    defs = [
        ("HORNER_INIT2_ANT",
         Spec(body=(_u0 * C0 + C1) * _u0,
              reference=lambda in0, in1, s0, s1, imm2:
              (_cl(in0) * s0 + s1) * _cl(in0))),
        ("HORNER2_ANT",
         Spec(body=((Src0 + C0) * _u1 + C1) * _u1,
              reference=lambda in0, in1, s0, s1, imm2:
              ((_f(in0) + s0) * _cl(in1) + s1) * _cl(in1))),
        ("HORNER3N_ANT",
         Spec(body=(((Src0 + C0) * _u1 + C1) * _u1 + C2) * _u1,
              reference=lambda in0, in1, s0, s1, imm2:
              (((_f(in0) + s0) * _cl(in1) + s1) * _cl(in1) + imm2) * _cl(in1))),
    ]
    ops = {}
    for name, spec in defs:
        if name in dops._SUB_OPCODE_FOR_NAME:
            ops[name] = next(o for o in dops.OPS if o.name == name)
            continue
        op = dops.DveOp(name, spec, subdim=False, uops_sha={})
        row = dops._CUSTOM_DVE_ROW_BASE + len(dops.OPS)
        assert row < 0x20, "custom-DVE opcode rows exhausted"
        dops.OPS.append(op)
        dops._SUB_OPCODE_FOR_NAME[name] = row
        dops.CUSTOM_DVE_SPECS[name] = spec
        su = DveOpSpec(name=name, opcode=row, uops=lower(spec, ver="v3"),
                       rd1_en=_has_src1(spec))
        object.__setattr__(op, "uops_sha", {"v3": su.sha("v3")})
        ops[name] = op
    _DVE_OPS = ops
    return ops


_PROGRAM = None
_PROGRAM_KC = None


def _build_program(Kc):
    """Build + compile the (SPMD, per-core) Bass/Tile program once."""
    global _PROGRAM, _PROGRAM_KC
    if _PROGRAM is not None:
        assert _PROGRAM_KC == Kc, "program cached with different Kc"
        return _PROGRAM

    from contextlib import ExitStack
    import concourse.bacc as bacc
    import concourse.tile as tile
    import concourse.mybir as mybir
    from concourse._compat import axon_active

    OPS = _register_dve_ops()
    f32 = mybir.dt.float32
    f16 = mybir.dt.float16
    Act = mybir.ActivationFunctionType

    nc = bacc.Bacc(
        "TRN2",
        target_bir_lowering=False,
        debug=not axon_active(),
        enable_asserts=True,
        num_devices=NCORES,
    )
    xt = nc.dram_tensor("xt", [NPAD, RPC], f16, kind="ExternalInput").ap()
    wm = nc.dram_tensor("wm", [NPAD, NREP * NPAD], f16, kind="ExternalInput").ap()
    cstf = nc.dram_tensor("cstf", [NPAD, 10], f32, kind="ExternalInput").ap()
    out = nc.dram_tensor("out", [NPAD, F], f16, kind="ExternalOutput").ap()

    with tile.TileContext(nc) as tc, ExitStack() as ctx:
        cpool = ctx.enter_context(tc.tile_pool(name="consts", bufs=1))
        xpool = ctx.enter_context(tc.tile_pool(name="xin", bufs=NBLK))
        ppool = ctx.enter_context(tc.tile_pool(name="ps", bufs=4, space="PSUM"))
        bpool = ctx.enter_context(tc.tile_pool(name="bb", bufs=2))
        ypool = ctx.enter_context(tc.tile_pool(name="yy", bufs=2))

        wm_sb = cpool.tile([NPAD, NREP * NPAD], f16)
        nc.scalar.dma_start(wm_sb[:], wm[:])
        cst_sb = cpool.tile([NPAD, 10], f32)
        nc.scalar.dma_start(cst_sb[:], cstf[:])

        scale_ap = cst_sb[:, 0:1]
        a0_ap = cst_sb[:, 1:2]
        hs = [cst_sb[:, 2 + i:3 + i] for i in range(8)]

        # input loads: 4 big transfers alternating the two HWDGE rings
        xts = []
        for j in range(NBLK):
            xt_t = xpool.tile([NPAD, NREP * BLKW], f16, tag="xin", name=f"x{j}")
            eng = nc.sync if j % 2 == 0 else nc.scalar
            eng.dma_start(xt_t[:], xt[:, j * NREP * BLKW:(j + 1) * NREP * BLKW])
            xts.append(xt_t)

        for j in range(NBLK):
            ps = ppool.tile([NPAD, BLKW], f32, tag="ps")
            for r in range(NREP):
                nc.tensor.matmul(
                    ps[:],
                    wm_sb[:, r * NPAD:(r + 1) * NPAD],
                    xts[j][:, r * BLKW:(r + 1) * BLKW],
                    start=(r == 0),
                    stop=(r == NREP - 1),
                )
            # degree-9 modified Horner straight out of PSUM (t = ps, clamped
            # inside each fused op); coefficient order [a9,a8 | a7,a6,(Kc) | a4,a3 | a2,a1]
            b_t = bpool.tile([NPAD, BLKW], f32, tag="bb")
            v = nc.vector
            v._custom_dve(OPS["HORNER_INIT2_ANT"], out=b_t[:], in0=ps[:],
                          s0=hs[0], s1=hs[1])
            v._custom_dve(OPS["HORNER3N_ANT"], out=b_t[:], in0=b_t[:],
                          in1=ps[:], s0=hs[2], s1=hs[3], imm2=Kc)
            v._custom_dve(OPS["HORNER2_ANT"], out=b_t[:], in0=b_t[:],
                          in1=ps[:], s0=hs[4], s1=hs[5])
            v._custom_dve(OPS["HORNER2_ANT"], out=b_t[:], in0=b_t[:],
                          in1=ps[:], s0=hs[6], s1=hs[7])
            # y = tanh(b/lambda + a_0), f32 -> fp16
            y_t = ypool.tile([NPAD, BLKW], f16, tag="yy")
            nc.scalar.activation(y_t[:], b_t[:], Act.Tanh,
                                 bias=a0_ap, scale=scale_ap)
            eng = nc.sync if j % 2 == 0 else nc.scalar
            eng.dma_start(out[:, j * BLKW:(j + 1) * BLKW], y_t[:])

    nc.compile()
    _PROGRAM = nc
    _PROGRAM_KC = Kc
    return nc


LAST_EXEC_NS = None


def kernel(**inputs) -> np.ndarray:
    global LAST_EXEC_NS
    from concourse.bass_utils import run_bass_kernel_spmd

    Wm, cstf, Kc = _fit_tables(inputs)
    X16 = np.asarray(inputs["wave_features"], np.float32).reshape(
        B * S, NW).astype(np.float16)

    in_maps = []
    for k in range(NCORES):
        in_maps.append({"xt": _prep_xt(X16, k), "wm": Wm, "cstf": cstf})

    nc = _build_program(Kc)
    trace = os.environ.get("BASS_KERNEL_PROFILE") == "1"
    res = run_bass_kernel_spmd(nc, in_maps, core_ids=list(range(NCORES)),
                               trace=trace)
    LAST_EXEC_NS = res.exec_time_ns
    # unshard: [128, 2048] fp16 feature-major -> [16384, 13] f32 row-major
    outs = []
    for k in range(NCORES):
        buf = np.asarray(res.results[k]["out"])[:NP_USED].astype(np.float32)
        outs.append(buf.reshape(NREP, NB, F).transpose(0, 2, 1).reshape(RPC, NB))
    return np.concatenate(outs, axis=0).reshape(B, S, NB)
